# revision 8
# baseline (speedup 1.0000x reference)
"""Trainium2 Bass kernel for fused causal GQA attention block.

Reference computation (B=1, S=2048, H=4096, NH=32, NKV=8, HD=128):
    qkv = hs @ w_attn.T; rope(q), rope(k); causal GQA attention;
    out @ w_proj.T

Sharding (8 cores, tensor parallel): core i owns kv-group i = rows
[i*768, (i+1)*768) of w_attn (4 q heads + 1 k + 1 v head) and rows
[i*512, (i+1)*512) of w_proj.

All heavy compute runs in bf16 (fp32 PSUM accumulation): full-rate PE
with fast weight load, half the DMA/SBUF/collective traffic of fp32.

Schedule: for each 512-seq block nb: QKV GEMM (2 passes of 3 qkv row
tiles over streamed hs slabs) -> rope(q,k) on DVE + V transpose via
XBAR DMA -> attention chunk nb (4 q-blocks of 128, all 4 heads fused
into the 512-wide free dim; causal mask added in PSUM; exp on ACT
pipelined 2 deep against the score matmuls) -> AllGather of the
block's attention output (bf16) fired immediately so all 4 collectives
hide under compute. Final c_proj consumes gathered chunks.
"""

import sys

sys.path.insert(0, "/opt/trn_rl_repo")

import ml_dtypes
import numpy as np

import concourse.bass as bass
import concourse.tile as tile
from concourse import bacc, mybir
from concourse.bass_utils import run_bass_kernel_spmd

F32 = mybir.dt.float32
BF16 = mybir.dt.bfloat16
BF16NP = ml_dtypes.bfloat16

B, S, H = 1, 2048, 4096
NH, NKV, HD = 32, 8, 128
GROUP = NH // NKV  # 4
SCALE = 0.08838834764831845
NCORES = 8

M_SHARD = (GROUP + 2) * HD  # 768 rows of w_attn per core
P_SHARD = H // NCORES  # 512 rows of w_proj per core

KC = H // 128  # 32 contraction chunks of the model dim
NB = S // 512  # 4 seq blocks of 512
MT = M_SHARD // 128  # 6 row tiles of qkv_t
QT = S // 128  # 16 q blocks of 128
MASKBIG = -600.0  # additive causal mask (-600 * SCALE ~ -53 before exp)


def build_module() -> bass.Bass:
    nc = bacc.Bacc(
        "TRN2",
        target_bir_lowering=False,
        debug=False,
        num_devices=NCORES,
    )

    hs_t = nc.dram_tensor("hs_t", [H, S], BF16, kind="ExternalInput")
    wa_t = nc.dram_tensor("wa_t", [H, M_SHARD], BF16, kind="ExternalInput")
    wp_t = nc.dram_tensor("wp_t", [H, P_SHARD], BF16, kind="ExternalInput")
    cos_t = nc.dram_tensor("cos_t", [HD, S], BF16, kind="ExternalInput")
    sin_t = nc.dram_tensor("sin_t", [HD, S], BF16, kind="ExternalInput")
    rot_t = nc.dram_tensor("rot_t", [HD, HD], BF16, kind="ExternalInput")
    masks_in = nc.dram_tensor("masks_in", [128, 512], BF16, kind="ExternalInput")
    ones_in = nc.dram_tensor("ones_in", [128, 128], BF16, kind="ExternalInput")
    ident_in = nc.dram_tensor("ident_in", [128, 128], BF16, kind="ExternalInput")
    y_out = nc.dram_tensor("y", [S, P_SHARD], F32, kind="ExternalOutput")

    # per-seq-chunk collective buffers (bf16 halves the wire bytes)
    ag_ins = [
        nc.dram_tensor(f"ag_in{i}", [GROUP * HD, 512], BF16, kind="Internal")
        for i in range(NB)
    ]
    ag_outs = [
        nc.dram_tensor(
            f"ag_out{i}", [H, 512], BF16, kind="Internal", addr_space="Shared"
        )
        for i in range(NB)
    ]

    # DRAM views with 128-partition tiling of the contraction axis
    hs_v = hs_t[:].rearrange("(ko p) n -> p ko n", p=128)  # [128, 32, 2048]
    wa_v = wa_t[:].rearrange("(ko p) m -> p ko m", p=128)  # [128, 32, 768]
    wp_v = wp_t[:].rearrange("(ko p) m -> p ko m", p=128)  # [128, 32, 512]
    ag_rd = [a[:].rearrange("(ko p) n -> p ko n", p=128) for a in ag_outs]
    # write view: feature row h*128+d <- at[d (part), (h, qq)]
    ag_wr = [a[:].rearrange("(h d) s -> d h s", h=GROUP) for a in ag_ins]

    with tile.TileContext(nc) as tc:
        # ---------- persistent pools ----------
        qkv_pool = tc.alloc_tile_pool(name="qkv", bufs=1)
        w_pool = tc.alloc_tile_pool(name="w", bufs=1)
        const_pool = tc.alloc_tile_pool(name="consts", bufs=1)
        vnat_pool = tc.alloc_tile_pool(name="vnat", bufs=1)
        rope_pool = tc.alloc_tile_pool(name="rope", bufs=2)
        pt_pool = tc.alloc_tile_pool(name="pt", bufs=3)
        attn_pool = tc.alloc_tile_pool(name="attn", bufs=2)
        psST = tc.alloc_tile_pool(name="psST", bufs=2, space="PSUM")
        psLO = tc.alloc_tile_pool(name="psLO", bufs=1, space="PSUM")
        hs_pool = tc.alloc_tile_pool(name="hs", bufs=2)
        psA = tc.alloc_tile_pool(name="psA", bufs=1, space="PSUM")

        qkv_sb = qkv_pool.tile([128, MT, S], BF16)  # 24KB/part
        wa_sb = w_pool.tile([128, KC, M_SHARD], BF16)  # 48KB/part
        v_nat = vnat_pool.tile([128, QT, HD], BF16)  # 4KB/part

        ones_sb = const_pool.tile([128, 128], BF16, tag="ones")
        ident_sb = const_pool.tile([128, 128], BF16, tag="ident")
        rot_sb = const_pool.tile([128, HD], BF16, tag="rot")
        masks_sb = const_pool.tile([128, 512], BF16, tag="masks")
        cos_sb = const_pool.tile([128, S], BF16, tag="cos")
        sin_sb = const_pool.tile([128, S], BF16, tag="sin")

        # ---------- preloads ----------
        # wa group-0 columns on scalar, group-1 on gpsimd, so phase A can
        # start ASAP and both halves stream in parallel
        mc0, mc1 = slice(0, 384), slice(384, 768)
        for lo, hi in zip([0, 2, 8, 16, 24], [2, 8, 16, 24, 32]):
            nc.scalar.dma_start(
                out=wa_sb[:, lo:hi, mc0], in_=wa_v[:, lo:hi, mc0]
            )
        nc.gpsimd.dma_start(out=cos_sb, in_=cos_t[:])
        nc.gpsimd.dma_start(out=sin_sb, in_=sin_t[:])
        nc.gpsimd.dma_start(out=ones_sb, in_=ones_in[:])
        nc.gpsimd.dma_start(out=ident_sb, in_=ident_in[:])
        nc.gpsimd.dma_start(out=rot_sb, in_=rot_t[:])
        nc.gpsimd.dma_start(out=masks_sb, in_=masks_in[:])
        for lo, hi in zip([0, 8, 16, 24], [8, 16, 24, 32]):
            nc.gpsimd.dma_start(
                out=wa_sb[:, lo:hi, mc1], in_=wa_v[:, lo:hi, mc1]
            )
        kT = qkv_sb[:, GROUP, :]

        for nb in range(NB):
            sl = slice(nb * 512, (nb + 1) * 512)

            # ---------- phase A: qkv_t[:, :, nb] = wa_shard @ hs[nb].T ----
            hs_nb = hs_pool.tile([128, KC, 512], BF16, name="hs_nb")
            bounds = [0, 2, 8, 16, 24, 32] if nb == 0 else [0, 8, 16, 24, 32]
            for lo, hi in zip(bounds, bounds[1:]):
                nc.sync.dma_start(
                    out=hs_nb[:, lo:hi, :], in_=hs_v[:, lo:hi, sl]
                )
            for g in range(3):
                ms = [2 * g, 2 * g + 1]
                psums = [
                    psA.tile([128, 512], F32, tag=f"a{i}", name=f"psA{i}")
                    for i in range(2)
                ]
                for k in range(KC):
                    for i, m in enumerate(ms):
                        nc.tensor.matmul(
                            psums[i],
                            lhsT=wa_sb[:, k, m * 128 : (m + 1) * 128],
                            rhs=hs_nb[:, k, :],
                            start=(k == 0),
                            stop=(k == KC - 1),
                        )
                for i, m in enumerate(ms):
                    nc.vector.tensor_copy(out=qkv_sb[:, m, sl], in_=psums[i])

            # ---------- rope on q0..q3 and k for this block, in place ----
            for t in range(GROUP + 1):
                x = qkv_sb[:, t, sl]
                rpp = psST.tile([128, 2, 512], F32, tag="st", name="rp")
                rp = rpp[:, 0, :]
                nc.tensor.matmul(rp, lhsT=rot_sb, rhs=x, start=True, stop=True)
                rs = rope_pool.tile([128, 512], BF16, name="rs")
                nc.vector.tensor_mul(rs, rp, sin_sb[:, sl])
                nc.vector.tensor_mul(x, x, cos_sb[:, sl])
                nc.vector.tensor_add(x, x, rs)

            # ---------- v natural layout via XBAR transpose DMA ----------
            for jj in range(4):
                j = nb * 4 + jj
                nc.sync.dma_start_transpose(
                    out=v_nat[:, j, :],
                    in_=qkv_sb[:, GROUP + 1, j * 128 : (j + 1) * 128],
                )

            # ---------- attention chunk nb: q blocks of 128, heads fused -
            for qi in range(nb * 4, nb * 4 + 4):
                rhs_q = qkv_sb[:, 0:GROUP, qi * 128 : (qi + 1) * 128]
                njt = qi + 1
                l_ps = psLO.tile([128, 512], F32, tag="l", name="l_ps")
                o_ps = psLO.tile([128, 512], F32, tag="o", name="o_ps")

                def emit_lo(j, pt):
                    nc.tensor.matmul(
                        l_ps,
                        lhsT=ones_sb,
                        rhs=pt,
                        start=(j == 0),
                        stop=(j == njt - 1),
                    )
                    nc.tensor.matmul(
                        o_ps,
                        lhsT=v_nat[:, j, :],
                        rhs=pt,
                        start=(j == 0),
                        stop=(j == njt - 1),
                    )

                def emit_st(stph, j):
                    diag = j == qi
                    nc.tensor.matmul(
                        stph,
                        lhsT=kT[:, j * 128 : (j + 1) * 128],
                        rhs=rhs_q,
                        start=True,
                        stop=not diag,
                    )
                    if diag:  # add -600 above the in-block diagonal
                        nc.tensor.matmul(
                            stph,
                            lhsT=ident_sb,
                            rhs=masks_sb,
                            start=False,
                            stop=True,
                        )

                pend = []
                for j0 in range(0, njt, 2):
                    j1 = j0 + 1 if j0 + 1 < njt else None
                    w = 2 if j1 is not None else 1
                    stp = psST.tile([128, 2, 512], F32, tag="st", name="st")
                    emit_st(stp[:, 0, :], j0)
                    if j1 is not None:
                        emit_st(stp[:, 1, :], j1)
                    ptp = pt_pool.tile([128, 2, 512], BF16, name="pt")
                    nc.scalar.activation(
                        out=ptp[:, :w, :],
                        in_=stp[:, :w, :],
                        func=mybir.ActivationFunctionType.Exp,
                        scale=SCALE,
                    )
                    pend.append((j0, j1, ptp))
                    if len(pend) > 1:
                        pj0, pj1, pptp = pend.pop(0)
                        emit_lo(pj0, pptp[:, 0, :])
                        if pj1 is not None:
                            emit_lo(pj1, pptp[:, 1, :])
                for pj0, pj1, pptp in pend:
                    emit_lo(pj0, pptp[:, 0, :])
                    if pj1 is not None:
                        emit_lo(pj1, pptp[:, 1, :])

                linv = attn_pool.tile([128, 512], F32, tag="linv", name="linv")
                nc.vector.reciprocal_approx_fast(linv, l_ps)
                at = attn_pool.tile([128, 512], BF16, tag="at", name="at")
                nc.vector.tensor_mul(at, o_ps, linv)
                qsub = qi % 4
                nc.gpsimd.dma_start(
                    out=ag_wr[nb][:, :, qsub * 128 : (qsub + 1) * 128], in_=at
                )

            # ---- seq-chunked AllGather (overlaps all remaining compute)
            nc.gpsimd.collective_compute(
                "AllGather",
                mybir.AluOpType.bypass,
                replica_groups=[list(range(NCORES))],
                ins=[ag_ins[nb][:]],
                outs=[ag_outs[nb][:]],
            )

        psA.release()
        hs_pool.release()

        # ---------- c_proj: y[mt] = attnT[:, mt].T @ wp_shard ----------
        with (
            tc.tile_pool(name="wp", bufs=1) as wp_pool,
            tc.tile_pool(name="lh", bufs=2) as lh_pool,
            tc.tile_pool(name="ysb", bufs=2) as y_pool,
            tc.tile_pool(name="psC", bufs=2, space="PSUM") as psC,
        ):
            wp_sb = wp_pool.tile([128, KC, P_SHARD], BF16)  # 32KB/part
            for kk in range(0, KC, 8):
                nc.sync.dma_start(
                    out=wp_sb[:, kk : kk + 8, :], in_=wp_v[:, kk : kk + 8, :]
                )
            for c in range(NB):
                for sub in range(4):
                    mt = c * 4 + sub
                    lh = lh_pool.tile([128, KC, 128], BF16, name="lh")
                    nc.sync.dma_start(
                        out=lh, in_=ag_rd[c][:, :, sub * 128 : (sub + 1) * 128]
                    )
                    yp = psC.tile([128, 512], F32, name="yp")
                    for k in range(KC):
                        nc.tensor.matmul(
                            yp,
                            lhsT=lh[:, k, :],
                            rhs=wp_sb[:, k, :],
                            start=(k == 0),
                            stop=(k == KC - 1),
                        )
                    ysb = y_pool.tile([128, P_SHARD], F32, name="ysb")
                    nc.scalar.activation(
                        out=ysb,
                        in_=yp,
                        func=mybir.ActivationFunctionType.Copy,
                    )
                    nc.gpsimd.dma_start(
                        out=y_out[mt * 128 : (mt + 1) * 128, :], in_=ysb
                    )

        for p in (
            psLO,
            psST,
            attn_pool,
            pt_pool,
            rope_pool,
            vnat_pool,
            const_pool,
            w_pool,
            qkv_pool,
        ):
            p.release()

    nc.compile()
    return nc


_CACHED = {}


def _get_module():
    if "nc" not in _CACHED:
        _CACHED["nc"] = build_module()
    return _CACHED["nc"]


def make_in_maps(hidden_states, w_attn, w_proj, rope_cos, rope_sin):
    hidden_states = np.asarray(hidden_states, dtype=np.float32)
    w_attn = np.asarray(w_attn, dtype=np.float32)
    w_proj = np.asarray(w_proj, dtype=np.float32)
    rope_cos = np.asarray(rope_cos, dtype=np.float32)
    rope_sin = np.asarray(rope_sin, dtype=np.float32)

    hs_t = np.ascontiguousarray(hidden_states.reshape(S, H).T).astype(BF16NP)
    cos_t = np.ascontiguousarray(rope_cos.T).astype(BF16NP)
    sin_t = np.ascontiguousarray(rope_sin.T).astype(BF16NP)

    # rotate-half as a matmul: rot(x) = R @ x for x in [HD, S] layout,
    # rot_t = R.T so that lhsT.T @ x = R @ x
    rot_t = np.zeros((HD, HD), dtype=np.float32)
    half = HD // 2
    rot_t[half + np.arange(half), np.arange(half)] = -1.0
    rot_t[np.arange(half), half + np.arange(half)] = 1.0
    rot_t = rot_t.astype(BF16NP)

    # additive causal mask for the diagonal 128x128 block, repeated for
    # the 4 fused heads: masks[k, h*128+qq] = MASKBIG iff qq < k
    kk_, qq_ = np.meshgrid(np.arange(128), np.arange(128), indexing="ij")
    m128 = np.where(qq_ < kk_, MASKBIG, 0.0).astype(np.float32)
    masks = np.tile(m128, (1, GROUP)).astype(BF16NP)

    ones = np.ones((128, 128), dtype=np.float32).astype(BF16NP)
    ident = np.eye(128, dtype=np.float32).astype(BF16NP)

    in_maps = []
    for i in range(NCORES):
        wa_sh = w_attn[i * M_SHARD : (i + 1) * M_SHARD, :]
        wp_sh = w_proj[i * P_SHARD : (i + 1) * P_SHARD, :]
        in_maps.append(
            {
                "hs_t": hs_t,
                "wa_t": np.ascontiguousarray(wa_sh.T).astype(BF16NP),
                "wp_t": np.ascontiguousarray(wp_sh.T).astype(BF16NP),
                "cos_t": cos_t,
                "sin_t": sin_t,
                "rot_t": rot_t,
                "masks_in": masks,
                "ones_in": ones,
                "ident_in": ident,
            }
        )
    return in_maps


def kernel(hidden_states, w_attn, w_proj, rope_cos, rope_sin, **_unused):
    nc = _get_module()
    in_maps = make_in_maps(hidden_states, w_attn, w_proj, rope_cos, rope_sin)
    res = run_bass_kernel_spmd(nc, in_maps, core_ids=list(range(NCORES)))

    out = np.empty((S, H), dtype=np.float32)
    for i in range(NCORES):
        out[:, i * P_SHARD : (i + 1) * P_SHARD] = res.results[i]["y"]
    return out.reshape(B, S, H)
